# revision 51
# baseline (speedup 1.0000x reference)
"""Trainium2 Bass kernel for nn_DeepEC_KAN (DeepEC conv->maxpool->BN->LN->KAN x2).

Data parallel over batch (256 -> 32 per core on 8 cores). Per core:
  - conv1d(K=4/8/16) as bf16 matmuls over a 6-tap-shifted patch; the 6 tap
    copies are loaded straight from HBM (overlapping windows of x) on the
    two HWDGE queues, 4-sample chunks, double buffered.
  - maxpool fused into one DVE tensor_tensor_reduce per conv (max of the
    two PSUM halves + max-reduce in a single 1x pass).
  - BN1..4 + conv bias folded into per-channel affine on host.
  - LayerNorm stats via ones-vector matmuls; KAN silu on ACT; cubic
    B-spline bases via batched DVE/GPSIMD ops (broadcast APs).
  - KAN contraction matmuls bf16 with samples as the stationary operand.
  - tail (LN+KAN) in two 16-sample halves; tail 1 is emitted in stages
    interleaved with conv chunks 4-5 so the PE queue never blocks.
  - PE kept HAM-warm with early warmup matmuls; ACT tables pre-loaded.
"""

import sys
import numpy as np
import ml_dtypes

sys.path.insert(0, "/opt/trn_rl_repo")

import concourse.bass as bass  # noqa: E402
import concourse.bacc as bacc  # noqa: E402
import concourse.tile as tile  # noqa: E402
from concourse import mybir  # noqa: E402
from concourse.bass_utils import run_bass_kernel_spmd  # noqa: E402

F32 = mybir.dt.float32
F16 = mybir.dt.float16
BF16 = mybir.dt.bfloat16
ALU = mybir.AluOpType
ACTF = mybir.ActivationFunctionType
AX = mybir.AxisListType

NCORES = 8
B = 256
BC = B // NCORES  # 32 samples per core
C = 21
L = 1000
LP = 1008
CH = 4                 # samples per patch chunk
NCHUNK = BC // CH      # 8 chunks
NH = BC // 2           # tail half-batch (16)
CONV_L = [997, 993, 985]
# (wconv row0, n rows, col offset, conv idx, first, last)
GROUPS = [
    (0, 84, 0, 0, True, True),      # conv1 taps 0-3
    (84, 126, 0, 1, True, False),   # conv2 taps 0-5
    (210, 42, 6, 1, False, True),   # conv2 taps 6-7
    (252, 126, 0, 2, True, False),  # conv3 taps 0-5
    (378, 126, 6, 2, False, False),  # conv3 taps 6-11
    (504, 84, 12, 2, False, True),  # conv3 taps 12-15
]
WCONV_ROWS = 588
NW1 = 21
NW2 = 28
D1OUT = 512
D2OUT = 229
W2PAD = 256
NEG_INF = -60000.0


def _emit_splines(nc, pool, x2d, W, tag, engA, engB, kvec, tick):
    """bs [128, 6, W] (bf16) <- cubic B-spline bases (x6) of x2d [128, W].

    engA carries the serial polynomial chain, engB the indicator/product
    side; both see only SBUF.
    """
    t = pool.tile([128, W], F16, tag=f"{tag}_t", name=f"{tag}_t")
    engA.tensor_scalar(out=t, in0=x2d, scalar1=1.5, scalar2=4.5,
                       op0=ALU.mult, op1=ALU.add)
    st = pool.tile([128, 10, W], F16, tag=f"{tag}_st", name=f"{tag}_st")
    nc.vector.tensor_tensor(out=st,
                            in0=t[:, None, :].broadcast_to([128, 10, W]),
                            in1=kvec[:, :, None].broadcast_to([128, 10, W]),
                            op=ALU.is_ge)
    tick()
    kf = pool.tile([128, W], F16, tag=f"{tag}_kf", name=f"{tag}_kf")
    stv = st.rearrange("p k w -> p w k")[:, :, 1:9]
    with nc.allow_low_precision(reason="sum of 8 indicator bits, exact fp16"):
        nc.vector.reduce_sum(out=kf, in_=stv, axis=AX.X)
    q = pool.tile([128, 9, W], F16, tag=f"{tag}_q", name=f"{tag}_q")
    engB.tensor_sub(q, st[:, 0:9], st[:, 1:10])
    u = pool.tile([128, W], F16, tag=f"{tag}_u", name=f"{tag}_u")
    engA.tensor_sub(u, t, kf)
    u2 = pool.tile([128, W], F16, tag=f"{tag}_u2", name=f"{tag}_u2")
    engA.tensor_mul(u2, u, u)
    u3 = pool.tile([128, W], F16, tag=f"{tag}_u3", name=f"{tag}_u3")
    engA.tensor_mul(u3, u2, u)
    tick()
    w1 = pool.tile([128, W], F16, tag=f"{tag}_w1", name=f"{tag}_w1")
    engA.tensor_scalar(out=w1, in0=u, scalar1=-1.0, scalar2=1.0,
                       op0=ALU.mult, op1=ALU.add)
    w2 = pool.tile([128, W], F16, tag=f"{tag}_w2", name=f"{tag}_w2")
    engA.tensor_mul(w2, w1, w1)
    D3 = pool.tile([128, W], F16, tag=f"{tag}_D3", name=f"{tag}_D3")
    engA.tensor_mul(D3, w2, w1)
    D0 = u3
    u26 = pool.tile([128, W], F16, tag=f"{tag}_u26", name=f"{tag}_u26")
    engA.tensor_scalar_mul(u26, u2, 6.0)
    D2 = pool.tile([128, W], F16, tag=f"{tag}_D2", name=f"{tag}_D2")
    nc.vector.scalar_tensor_tensor(out=D2, in0=u3, scalar=3.0, in1=u26,
                                   op0=ALU.mult, op1=ALU.subtract)
    engA.tensor_scalar_add(D2, D2, 4.0)
    tick()
    D1 = pool.tile([128, W], F16, tag=f"{tag}_D1", name=f"{tag}_D1")
    engA.tensor_add(D1, D0, D2)
    engA.tensor_add(D1, D1, D3)
    engA.tensor_scalar(out=D1, in0=D1, scalar1=-1.0, scalar2=6.0,
                       op0=ALU.mult, op1=ALU.add)
    Ds = [D0, D1, D2, D3]
    ms = []
    for j in range(4):
        m = pool.tile([128, 6, W], F16, tag=f"{tag}_m{j}", name=f"{tag}_m{j}")
        nc.vector.tensor_mul(m, q[:, j:j + 6],
                             Ds[j][:, None, :].broadcast_to([128, 6, W]))
        ms.append(m)
    tick()
    a01 = pool.tile([128, 6, W], F16, tag=f"{tag}_a01", name=f"{tag}_a01")
    engA.tensor_add(a01, ms[0], ms[1])
    a23 = pool.tile([128, 6, W], F16, tag=f"{tag}_a23", name=f"{tag}_a23")
    engB.tensor_add(a23, ms[2], ms[3])
    bs = pool.tile([128, 6, W], BF16, tag=f"{tag}_bs", name=f"{tag}_bs")
    engA.tensor_add(bs, a01, a23)
    return bs


def _build_program():
    nc = bacc.Bacc("TRN2", target_bir_lowering=False, debug=False,
                   num_devices=NCORES)
    xpad_d = nc.dram_tensor("xpad", [C, BC, LP], BF16,
                            kind="ExternalInput").ap()
    wconv_d = nc.dram_tensor("wconv", [WCONV_ROWS, 128], BF16,
                             kind="ExternalInput").ap()
    kconst_d = nc.dram_tensor("kconst", [128, 5, 96], F32,
                              kind="ExternalInput").ap()
    kvec_d = nc.dram_tensor("kvec", [128, 10], F16, kind="ExternalInput").ap()
    w1s_d = nc.dram_tensor("w1s", [128, NW1, D1OUT], BF16,
                           kind="ExternalInput").ap()
    w2s_d = nc.dram_tensor("w2s", [128, NW2, W2PAD], BF16,
                           kind="ExternalInput").ap()
    id32_d = nc.dram_tensor("id32", [32, 32], F32, kind="ExternalInput").ap()
    out_d = nc.dram_tensor("out", [BC, D2OUT], F32, kind="ExternalOutput").ap()

    with tile.TileContext(nc) as tc:
        with (
            tc.tile_pool(name="const", bufs=1) as const,
            tc.tile_pool(name="patches", bufs=2) as patches,
            tc.tile_pool(name="work", bufs=1) as work,
            tc.tile_pool(name="psconv", bufs=1, space="PSUM") as psconv,
            tc.tile_pool(name="psc1", bufs=3, space="PSUM") as psc1,
            tc.tile_pool(name="pstail", bufs=1, space="PSUM") as pstail,
        ):
            # ---- constants / warmup ----
            wu_l = const.tile([128, 128], BF16, tag="wu_l", name="wu_l")
            nc.vector.memset(wu_l, 0.125)
            wu_r = const.tile([128, 512], BF16, tag="wu_r", name="wu_r")
            nc.vector.memset(wu_r, 0.125)
            ones = const.tile([128, 128], F32, tag="ones", name="ones")
            nc.vector.memset(ones, 1.0)
            # HAM warm-up: keep PE busy while the first patch DMAs land.
            wups = pstail.tile([128, 512], F32, tag="big", name="wups")
            for i in range(10):
                nc.tensor.matmul(out=wups, lhsT=wu_l, rhs=wu_r,
                                 start=True, stop=True)
            # pre-load the ACT table set for Sigmoid (the only ACT function
            # used; Copy needs no table)
            wtab = const.tile([128, 1], F32, tag="wtab", name="wtab")
            nc.scalar.activation(out=wtab, in_=ones[:, 0:1], func=ACTF.Sigmoid)

            wc_tiles = []
            for gi, (r0, nr, _off, _cj, _f, _l) in enumerate(GROUPS):
                wt = const.tile([128, 128], BF16, tag=f"wc{gi}", name=f"wc{gi}")
                nc.sync.dma_start(out=wt[0:nr, :], in_=wconv_d[r0:r0 + nr, :])
                wc_tiles.append(wt)
            kc = const.tile([128, 5, 96], F32, tag="kc", name="kc")
            nc.sync.dma_start(out=kc, in_=kconst_d)
            kvec = work.tile([128, 10], F16, tag="kvec", name="kvec_l")
            nc.sync.dma_start(out=kvec, in_=kvec_d)
            id32 = const.tile([32, 32], F32, tag="id32", name="id32")
            nc.sync.dma_start(out=id32, in_=id32_d)
            w1s = const.tile([128, NW1, D1OUT], BF16, tag="w1s", name="w1s")
            nc.sync.dma_start(out=w1s, in_=w1s_d)
            w2s = const.tile([128, NW2, W2PAD], BF16, tag="w2s", name="w2s")
            nc.sync.dma_start(out=w2s, in_=w2s_d)

            # mraw rows: conv1-half-a, conv1-half-b, conv2, conv3
            mraw = work.tile([128, 4, BC], F32, tag="mraw", name="mraw")
            kc3 = kc.rearrange("p i (j b) -> p i j b", j=3)
            # conv2+conv3 maxpool: ACT casts PSUM->SBUF fp16 into a padded
            # [128, 2, 1024] tile; DVE folds both at 2x fp16 rate into a
            # per-chunk buffer; one batched fold+reduce per chunk.
            # conv1 is reduced straight from PSUM on DVE in two phases
            # (two single-bank psum generations -> no PE<->DVE convoy).
            cpBs = []
            for par in range(2):
                cpB = work.tile([128, 2, 1024], F16, tag=f"cpB{par}",
                                name=f"cpB{par}")
                nc.vector.memset(cpB, NEG_INF)
                cpBs.append(cpB)
            ckbufs = []
            for par in range(2):
                ckb = work.tile([128, 2, CH, 256], F16, tag=f"ckb{par}",
                                name=f"ckb{par}")
                ckbufs.append(ckb)

            def emit_tail_stages(b0, hx):
                """Return [s0..s3] closures for the tail over samples
                [b0, b0+NH). hx selects engine balance (0: conv still
                running -> gpsimd-heavy; 1: end of kernel -> mixed)."""
                nb = NH
                W1W = 3 * nb
                W2W = 4 * nb
                sfx = f"h{hx}"
                # tail1 overlaps the conv stream: put the affine chain on
                # GPSIMD and split spline work DVE/GPSIMD. tail2 runs at
                # the end when DVE is free: everything on DVE for minimum
                # chain latency.
                ge = nc.gpsimd if hx == 0 else nc.vector
                engB = nc.gpsimd if hx == 0 else nc.vector
                engA = nc.vector
                st_ = {}

                def tick():
                    # dependency-bound LDWEIGHTS: executes as the spline
                    # chain progresses, keeping HAM warm through the tail-2
                    # end chain without real matmul work
                    if hx == 1:
                        nc.tensor.ldweights(weights=wu_l)

                def keepwarm(n):
                    # dummy matmuls keep HAM at 8/8 while the PE waits for
                    # the tail's DVE/GPSIMD chains (tail 2 only; during
                    # tail 1 the conv stream keeps the PE busy)
                    if hx == 0:
                        return
                    wk = pstail.tile([128, 64], F32, tag="big",
                                     name=f"wk{sfx}")
                    for _ in range(n):
                        nc.tensor.matmul(out=wk, lhsT=wu_l,
                                         rhs=wu_r[:, 0:64],
                                         start=True, stop=True)

                def s_aff():
                    # merge conv1's two partial maxima into row 1 in place
                    nc.vector.tensor_max(mraw[:, 1, b0:b0 + nb],
                                         mraw[:, 0, b0:b0 + nb],
                                         mraw[:, 1, b0:b0 + nb])
                    mrh = mraw[:, 1:4, b0:b0 + nb]
                    kch = kc3[:, :, :, b0:b0 + nb]
                    h96 = work.tile([128, 3, nb], F32, tag=f"h96{sfx}",
                                    name=f"h96{sfx}")
                    ge.tensor_add(h96, mrh, kch[:, 0])
                    ge.tensor_scalar_max(h96, h96, 0.0)
                    ge.tensor_mul(h96, h96, kch[:, 1])
                    ge.tensor_add(h96, h96, kch[:, 2])
                    sq96 = work.tile([128, 3, nb], F32, tag=f"sq96{sfx}",
                                     name=f"sq96{sfx}")
                    ge.tensor_mul(sq96, h96, h96)
                    st_["h96"] = h96
                    st_["sq96"] = sq96
                    st_["kch"] = kch
                    keepwarm(4)

                def s_ln():
                    h96, sq96, kch = st_["h96"], st_["sq96"], st_["kch"]
                    psLN = pstail.tile([1, 4 * W1W], F32, tag="big",
                                       name=f"psLN{sfx}")
                    nc.tensor.matmul(out=psLN[0:1, 0:W1W], lhsT=ones[:, 0:1],
                                     rhs=h96, start=True, stop=True)
                    nc.tensor.matmul(out=psLN[0:1, W1W:2 * W1W],
                                     lhsT=ones[:, 0:1], rhs=sq96,
                                     start=True, stop=True)
                    sums = work.tile([1, 2, nb], F32, tag=f"sums{sfx}",
                                     name=f"sums{sfx}")
                    psLNv = psLN[0:1, 0:2 * W1W].rearrange(
                        "p (x j b) -> p x b j", x=2, j=3)
                    nc.vector.reduce_sum(out=sums[0:1, 0], in_=psLNv[0:1, 0],
                                         axis=AX.X)
                    nc.vector.reduce_sum(out=sums[0:1, 1], in_=psLNv[0:1, 1],
                                         axis=AX.X)
                    muinv = work.tile([1, 2, nb], F32, tag=f"muinv{sfx}",
                                      name=f"muinv{sfx}")
                    nc.vector.tensor_scalar_mul(muinv[0:1, 0], sums[0:1, 0],
                                                1.0 / 384)
                    msq = work.tile([1, nb], F32, tag=f"msq{sfx}",
                                    name=f"msq{sfx}")
                    nc.vector.tensor_mul(msq, muinv[0:1, 0], muinv[0:1, 0])
                    var = work.tile([1, nb], F32, tag=f"var{sfx}",
                                    name=f"var{sfx}")
                    nc.vector.scalar_tensor_tensor(out=var, in0=sums[0:1, 1],
                                                   scalar=1.0 / 384, in1=msq,
                                                   op0=ALU.mult,
                                                   op1=ALU.subtract)
                    nc.vector.tensor_scalar_add(var, var, 1e-5)
                    # rsqrt(var) via bit-trick + 2 Newton steps, all on DVE
                    # (ACT Sqrt would force a table-set reload between the
                    # tail's Sigmoid uses: ~2.7us each on the critical path)
                    yi = work.tile([1, nb], mybir.dt.int32, tag=f"yi{sfx}",
                                   name=f"yi{sfx}")
                    nc.vector.tensor_scalar(out=yi, in0=var.bitcast(
                        mybir.dt.int32), scalar1=1, scalar2=None,
                        op0=ALU.arith_shift_right)
                    nc.vector.tensor_scalar(out=yi, in0=yi, scalar1=-1,
                                            scalar2=0x5f3759df, op0=ALU.mult,
                                            op1=ALU.add)
                    y = work.tile([1, nb], F32, tag=f"y{sfx}",
                                  name=f"y{sfx}")
                    nc.vector.tensor_copy(y, yi.bitcast(F32))
                    vh = work.tile([1, nb], F32, tag=f"vh{sfx}",
                                   name=f"vh{sfx}")
                    nc.vector.tensor_scalar_mul(vh, var, 0.5)
                    yt = work.tile([1, nb], F32, tag=f"yt{sfx}",
                                   name=f"yt{sfx}")
                    for _ in range(2):
                        nc.vector.tensor_mul(yt, y, y)
                        nc.vector.tensor_mul(yt, yt, vh)
                        nc.vector.tensor_scalar(out=yt, in0=yt, scalar1=-1.0,
                                                scalar2=1.5, op0=ALU.mult,
                                                op1=ALU.add)
                        nc.vector.tensor_mul(y, y, yt)
                    nc.vector.tensor_copy(muinv[0:1, 1], y)
                    psB = pstail.tile([128, 2, nb], F32, tag="big",
                                      name=f"psB{sfx}")
                    nc.tensor.matmul(out=psB, lhsT=ones[0:1, :],
                                     rhs=muinv[0:1], start=True, stop=True)
                    muinvB = work.tile([128, 2, nb], F32, tag=f"muinvB{sfx}",
                                       name=f"muinvB{sfx}")
                    nc.scalar.copy(out=muinvB, in_=psB)
                    hn = work.tile([128, 3, nb], F32, tag=f"hn{sfx}",
                                   name=f"hn{sfx}")
                    nc.vector.tensor_sub(hn, h96,
                                         muinvB[:, 0, None, :].broadcast_to(
                                             [128, 3, nb]))
                    nc.vector.tensor_mul(hn, hn,
                                         muinvB[:, 1, None, :].broadcast_to(
                                             [128, 3, nb]))
                    ge.tensor_mul(hn, hn, kch[:, 3])
                    ge.tensor_add(hn, hn, kch[:, 4])
                    st_["hn"] = hn
                    keepwarm(6)

                def s_silu1():
                    hn = st_["hn"]
                    hn2d = hn.rearrange("p j b -> p (j b)")
                    sig = work.tile([128, W1W], F32, tag=f"sig{sfx}",
                                    name=f"sig{sfx}")
                    nc.scalar.activation(out=sig, in_=hn2d, func=ACTF.Sigmoid)
                    sil = work.tile([128, W1W], BF16, tag=f"sil{sfx}",
                                    name=f"sil{sfx}")
                    engA.tensor_mul(sil, sig, hn2d)
                    bs1 = _emit_splines(nc, work, hn2d, W1W, f"sp1{sfx}",
                                        engA, engB, kvec, tick)
                    st_["sil"] = sil
                    st_["bs1"] = bs1
                    keepwarm(14)

                def s_kan1():
                    sil, bs1 = st_["sil"], st_["bs1"]
                    psK1 = pstail.tile([nb, D1OUT], F32, tag="big",
                                       name=f"psK1{sfx}")
                    mi = 0
                    for j in range(3):
                        nc.tensor.matmul(out=psK1,
                                         lhsT=sil[:, j * nb:(j + 1) * nb],
                                         rhs=w1s[:, j], start=(mi == 0),
                                         stop=(mi == NW1 - 1))
                        mi += 1
                    for j in range(3):
                        for g in range(6):
                            nc.tensor.matmul(
                                out=psK1,
                                lhsT=bs1[:, g, j * nb:(j + 1) * nb],
                                rhs=w1s[:, 3 + j * 6 + g],
                                start=(mi == 0), stop=(mi == NW1 - 1))
                            mi += 1
                    h2s = work.tile([nb, D1OUT], F32, tag=f"h2s{sfx}",
                                    name=f"h2s{sfx}")
                    nc.scalar.copy(out=h2s, in_=psK1)
                    psT = pstail.tile([128, 4 * nb], F32, tag="big",
                                      name=f"psT{sfx}")
                    for j in range(4):
                        nc.tensor.transpose(out=psT[:, j * nb:(j + 1) * nb],
                                            in_=h2s[:, j * 128:(j + 1) * 128],
                                            identity=id32[0:nb, 0:nb])
                    h2T = work.tile([128, 4 * nb], F32, tag=f"h2T{sfx}",
                                    name=f"h2T{sfx}")
                    nc.scalar.copy(out=h2T, in_=psT)
                    st_["h2T"] = h2T
                    keepwarm(4)

                def s_silu2():
                    h2T = st_["h2T"]
                    sig2 = work.tile([128, W2W], F32, tag=f"sig2{sfx}",
                                     name=f"sig2{sfx}")
                    nc.scalar.activation(out=sig2, in_=h2T, func=ACTF.Sigmoid)
                    sil2 = work.tile([128, W2W], BF16, tag=f"sil2{sfx}",
                                     name=f"sil2{sfx}")
                    engA.tensor_mul(sil2, sig2, h2T)
                    bs2 = _emit_splines(nc, work, h2T, W2W, f"sp2{sfx}",
                                        engA, engB, kvec, tick)
                    st_["sil2"] = sil2
                    st_["bs2"] = bs2
                    keepwarm(14)

                def s_kan2():
                    sil2, bs2 = st_["sil2"], st_["bs2"]
                    psK2 = pstail.tile([nb, W2PAD], F32, tag="big",
                                       name=f"psK2{sfx}")
                    mi = 0
                    for j in range(4):
                        nc.tensor.matmul(out=psK2,
                                         lhsT=sil2[:, j * nb:(j + 1) * nb],
                                         rhs=w2s[:, j], start=(mi == 0),
                                         stop=(mi == NW2 - 1))
                        mi += 1
                    for j in range(4):
                        for g in range(6):
                            nc.tensor.matmul(
                                out=psK2,
                                lhsT=bs2[:, g, j * nb:(j + 1) * nb],
                                rhs=w2s[:, 4 + j * 6 + g],
                                start=(mi == 0), stop=(mi == NW2 - 1))
                            mi += 1
                    outS = work.tile([nb, D2OUT], F32, tag=f"outS{sfx}",
                                     name=f"outS{sfx}")
                    nc.scalar.copy(out=outS, in_=psK2[:, 0:D2OUT])
                    nc.sync.dma_start(out=out_d[b0:b0 + nb], in_=outS)

                return [s_aff, s_ln, s_silu1, s_kan1, s_silu2, s_kan2]

            tail1 = emit_tail_stages(0, 0)
            tail2 = emit_tail_stages(NH, 1)
            # (chunk, sample) emission points for tail1's stages, spaced so
            # each stage's inputs are ready when the fenced PE stream
            # reaches its matmuls
            t1_points = {(5, 0): tail1[0], (5, 2): tail1[1],
                         (6, 0): tail1[2], (6, 3): tail1[3],
                         (7, 1): tail1[4], (7, 3): tail1[5]}

            # ---- conv phase ----
            # deferred per-chunk fold/reduce closures, drained one per
            # sample so the DVE never sees a chunk-boundary burst
            pending_ck = []

            def emit_ck_stage(c):
                ckb = ckbufs[c % 2]
                fA = work.tile([128, 2, CH, 128], F16, tag="fA", name="fA")
                fB = work.tile([128, 2, CH, 64], F16, tag="fB", name="fB")
                fC = work.tile([128, 2, CH, 32], F16, tag="fC", name="fC")
                pending_ck.append(lambda: nc.vector.tensor_max(
                    fA, ckb[:, :, :, 0:128], ckb[:, :, :, 128:256]))
                pending_ck.append(lambda: nc.vector.tensor_max(
                    fB, fA[:, :, :, 0:64], fA[:, :, :, 64:128]))
                pending_ck.append(lambda: nc.vector.tensor_max(
                    fC, fB[:, :, :, 0:32], fB[:, :, :, 32:64]))
                pending_ck.append(lambda: nc.vector.reduce_max(
                    out=mraw[:, 2:4, c * CH:(c + 1) * CH].unsqueeze(-1),
                    in_=fC, axis=AX.X))

            for c in range(NCHUNK):
                ptile = patches.tile([128, CH, LP], BF16, name="ptile")
                for s in range(6):
                    nc.gpsimd.dma_start(
                        out=ptile[s * C:(s + 1) * C, :, 0:LP - s],
                        in_=xpad_d[:, c * CH:(c + 1) * CH, s:LP])
                ckb = ckbufs[c % 2]
                for bi in range(CH):
                    stage = t1_points.get((c, bi))
                    if stage is not None:
                        stage()
                    b = c * CH + bi
                    pc = [None,
                          psconv.tile([128, 1024], F32, tag="pc1",
                                      name="pc1"),
                          psconv.tile([128, 1024], F32, tag="pc2",
                                      name="pc2")]
                    # conv1 phases first (their DVE reduces are the only
                    # PSUM->DVE drains; issuing them early gives the
                    # conv1 banks ~3 sample periods of slack), conv3
                    # before conv2 (its ACT copy starts first)
                    pc0a = psc1.tile([128, 512], F32, name="pc0")
                    nc.tensor.matmul(out=pc0a, lhsT=wc_tiles[0][0:84, :],
                                     rhs=ptile[0:84, bi, 0:512],
                                     start=True, stop=True)
                    pc0b = psc1.tile([128, 512], F32, name="pc0")
                    nc.tensor.matmul(out=pc0b[:, 0:485],
                                     lhsT=wc_tiles[0][0:84, :],
                                     rhs=ptile[0:84, bi, 512:CONV_L[0]],
                                     start=True, stop=True)
                    for gi in (3, 4, 5, 1, 2):
                        r0, nr, off, cj, first, last = GROUPS[gi]
                        for (n0, n1) in ((0, 512), (512, CONV_L[cj])):
                            nc.tensor.matmul(
                                out=pc[cj][:, n0:n1],
                                lhsT=wc_tiles[gi][0:nr, :],
                                rhs=ptile[0:nr, bi, off + n0:off + n1],
                                start=first, stop=last,
                            )
                    nc.vector.reduce_max(out=mraw[:, 0, b:b + 1],
                                         in_=pc0a, axis=AX.X)
                    nc.vector.reduce_max(out=mraw[:, 1, b:b + 1],
                                         in_=pc0b[:, 0:485], axis=AX.X)
                    if c == 0:
                        # startup: ACT is loading tables; drain conv2/3
                        # straight on the (idle) DVE instead
                        nc.vector.reduce_max(out=mraw[:, 2, b:b + 1],
                                             in_=pc[1][:, 0:CONV_L[1]],
                                             axis=AX.X)
                        nc.vector.reduce_max(out=mraw[:, 3, b:b + 1],
                                             in_=pc[2][:, 0:CONV_L[2]],
                                             axis=AX.X)
                    else:
                        cpB = cpBs[b % 2]
                        nc.scalar.copy(out=cpB[:, 1, 0:CONV_L[2]],
                                       in_=pc[2][:, 0:CONV_L[2]])
                        nc.scalar.copy(out=cpB[:, 0, 0:CONV_L[1]],
                                       in_=pc[1][:, 0:CONV_L[1]])
                        fB1 = work.tile([128, 2, 512], F16, tag="fB1",
                                        name="fB1")
                        nc.vector.tensor_max(fB1, cpB[:, :, 0:512],
                                             cpB[:, :, 512:1024])
                        nc.vector.tensor_max(ckb[:, :, bi, :],
                                             fB1[:, :, 0:256],
                                             fB1[:, :, 256:512])
                    if pending_ck:
                        pending_ck.pop(0)()
                    # scheduler-only fence: keep the per-sample interleaving
                    # (without it the scheduler clusters same-kind matmuls
                    # at chunk boundaries and ping-pongs PE<->DVE)
                    tc.no_sync_barrier()
                if c > 0:
                    emit_ck_stage(c)
            while pending_ck:
                pending_ck.pop(0)()
            for stage in tail2:
                tc.no_sync_barrier()
                stage()
    nc.compile()
    return nc


def _host_prep(inputs):
    f = np.float32
    bf = ml_dtypes.bfloat16
    x = np.asarray(inputs["x"], f)
    xT = np.ascontiguousarray(x.transpose(0, 2, 1))  # [B, 21, 1000]
    xTpad = np.zeros((B, C, LP), f)
    xTpad[:, :, :L] = xT
    xpads = []
    for i in range(NCORES):
        sh = xTpad[i * BC:(i + 1) * BC]  # [BC, 21, LP]
        xpads.append(np.ascontiguousarray(
            sh.transpose(1, 0, 2)).astype(bf))  # [21, BC, LP]

    def chunks(w, taps):
        return [np.ascontiguousarray(
            np.asarray(w, f)[:, :, t0:t1].transpose(2, 1, 0).reshape((t1 - t0) * C, 128))
            for t0, t1 in taps]

    wconv = np.concatenate(
        chunks(inputs["conv1_w"], [(0, 4)])
        + chunks(inputs["conv2_w"], [(0, 6), (6, 8)])
        + chunks(inputs["conv3_w"], [(0, 6), (6, 12), (12, 16)]), 0)

    def fold(p):
        g, bb, m, v = (np.asarray(inputs[p + s], f) for s in ("_g", "_b", "_m", "_v"))
        s = g / np.sqrt(v + 1e-5)
        return s, bb - m * s

    s1, t1 = fold("bn1")
    s2, t2 = fold("bn2")
    s3, t3 = fold("bn3")
    s4, t4 = fold("bn4")
    Sall = np.concatenate([s1, s2, s3]) * s4
    Tall = np.concatenate([t1, t2, t3]) * s4 + t4
    cb = np.concatenate([np.asarray(inputs["conv1_b"], f),
                         np.asarray(inputs["conv2_b"], f),
                         np.asarray(inputs["conv3_b"], f)])

    def expand(v):
        return np.repeat(np.asarray(v, f).reshape(3, 128).T[:, :, None], BC, 2)

    kconst = np.stack([expand(cb), expand(Sall), expand(Tall),
                       expand(np.asarray(inputs["ln_g"], f)),
                       expand(np.asarray(inputs["ln_b"], f))], 1)
    kconst = np.ascontiguousarray(kconst.reshape(128, 5, 96))

    kvec = np.broadcast_to(np.arange(10, dtype=f)[None, :], (128, 10))

    bw1 = np.asarray(inputs["base_w1"], f)
    sw1 = np.asarray(inputs["spline_w1"], f) / 6.0
    w1s = np.empty((128, NW1, D1OUT), f)
    for j in range(3):
        w1s[:, j, :] = bw1[:, j * 128:(j + 1) * 128].T
        for g in range(6):
            w1s[:, 3 + j * 6 + g, :] = sw1[:, j * 128:(j + 1) * 128, g].T
    bw2 = np.asarray(inputs["base_w2"], f)
    sw2 = np.asarray(inputs["spline_w2"], f) / 6.0
    w2s = np.zeros((128, NW2, W2PAD), f)
    for j in range(4):
        w2s[:, j, :D2OUT] = bw2[:, j * 128:(j + 1) * 128].T
        for g in range(6):
            w2s[:, 4 + j * 6 + g, :D2OUT] = sw2[:, j * 128:(j + 1) * 128, g].T

    shared = {
        "wconv": np.ascontiguousarray(wconv).astype(bf),
        "kconst": kconst,
        "kvec": np.ascontiguousarray(kvec).astype(np.float16),
        "w1s": np.ascontiguousarray(w1s).astype(bf),
        "w2s": np.ascontiguousarray(w2s).astype(bf),
        "id32": np.eye(32, dtype=f),
    }
    return shared, xpads


_NC_CACHE = None


def _get_nc():
    global _NC_CACHE
    if _NC_CACHE is None:
        _NC_CACHE = _build_program()
    return _NC_CACHE


def make_in_maps(inputs):
    shared, xpads = _host_prep(inputs)
    return [{**shared, "xpad": xpads[i]} for i in range(NCORES)]


def kernel(**inputs):
    nc = _get_nc()
    in_maps = make_in_maps(inputs)
    res = run_bass_kernel_spmd(nc, in_maps, list(range(NCORES)))
    return np.concatenate([res.results[i]["out"] for i in range(NCORES)], 0)


# revision 52
# speedup vs baseline: 1.0148x; 1.0148x over previous
"""Trainium2 Bass kernel for nn_DeepEC_KAN (DeepEC conv->maxpool->BN->LN->KAN x2).

Data parallel over batch (256 -> 32 per core on 8 cores). Per core:
  - conv1d(K=4/8/16) as bf16 matmuls over a 6-tap-shifted patch; the 6 tap
    copies are loaded straight from HBM (overlapping windows of x) on the
    two HWDGE queues, 4-sample chunks, double buffered.
  - maxpool fused into one DVE tensor_tensor_reduce per conv (max of the
    two PSUM halves + max-reduce in a single 1x pass).
  - BN1..4 + conv bias folded into per-channel affine on host.
  - LayerNorm stats via ones-vector matmuls; KAN silu on ACT; cubic
    B-spline bases via batched DVE/GPSIMD ops (broadcast APs).
  - KAN contraction matmuls bf16 with samples as the stationary operand.
  - tail (LN+KAN) in two 16-sample halves; tail 1 is emitted in stages
    interleaved with conv chunks 4-5 so the PE queue never blocks.
  - PE kept HAM-warm with early warmup matmuls; ACT tables pre-loaded.
"""

import sys
import numpy as np
import ml_dtypes

sys.path.insert(0, "/opt/trn_rl_repo")

import concourse.bass as bass  # noqa: E402
import concourse.bacc as bacc  # noqa: E402
import concourse.tile as tile  # noqa: E402
from concourse import mybir  # noqa: E402
from concourse.bass_utils import run_bass_kernel_spmd  # noqa: E402

F32 = mybir.dt.float32
F16 = mybir.dt.float16
BF16 = mybir.dt.bfloat16
ALU = mybir.AluOpType
ACTF = mybir.ActivationFunctionType
AX = mybir.AxisListType

NCORES = 8
B = 256
BC = B // NCORES  # 32 samples per core
C = 21
L = 1000
LP = 1008
CH = 4                 # samples per patch chunk
NCHUNK = BC // CH      # 8 chunks
NH = BC // 2           # tail half-batch (16)
CONV_L = [997, 993, 985]
# (wconv row0, n rows, col offset, conv idx, first, last)
GROUPS = [
    (0, 84, 0, 0, True, True),      # conv1 taps 0-3
    (84, 126, 0, 1, True, False),   # conv2 taps 0-5
    (210, 42, 6, 1, False, True),   # conv2 taps 6-7
    (252, 126, 0, 2, True, False),  # conv3 taps 0-5
    (378, 126, 6, 2, False, False),  # conv3 taps 6-11
    (504, 84, 12, 2, False, True),  # conv3 taps 12-15
]
WCONV_ROWS = 588
NW1 = 21
NW2 = 28
D1OUT = 512
D2OUT = 229
W2PAD = 256
NEG_INF = -60000.0


def _emit_splines(nc, pool, x2d, W, tag, engA, engB, kvec, tick):
    """bs [128, 6, W] (bf16) <- cubic B-spline bases (x6) of x2d [128, W].

    engA carries the serial polynomial chain, engB the indicator/product
    side; both see only SBUF.
    """
    t = pool.tile([128, W], F16, tag=f"{tag}_t", name=f"{tag}_t")
    engA.tensor_scalar(out=t, in0=x2d, scalar1=1.5, scalar2=4.5,
                       op0=ALU.mult, op1=ALU.add)
    st = pool.tile([128, 10, W], F16, tag=f"{tag}_st", name=f"{tag}_st")
    nc.vector.tensor_tensor(out=st,
                            in0=t[:, None, :].broadcast_to([128, 10, W]),
                            in1=kvec[:, :, None].broadcast_to([128, 10, W]),
                            op=ALU.is_ge)
    tick()
    kf = pool.tile([128, W], F16, tag=f"{tag}_kf", name=f"{tag}_kf")
    stv = st.rearrange("p k w -> p w k")[:, :, 1:9]
    with nc.allow_low_precision(reason="sum of 8 indicator bits, exact fp16"):
        nc.vector.reduce_sum(out=kf, in_=stv, axis=AX.X)
    q = pool.tile([128, 9, W], F16, tag=f"{tag}_q", name=f"{tag}_q")
    engB.tensor_sub(q, st[:, 0:9], st[:, 1:10])
    u = pool.tile([128, W], F16, tag=f"{tag}_u", name=f"{tag}_u")
    engA.tensor_sub(u, t, kf)
    u2 = pool.tile([128, W], F16, tag=f"{tag}_u2", name=f"{tag}_u2")
    engA.tensor_mul(u2, u, u)
    u3 = pool.tile([128, W], F16, tag=f"{tag}_u3", name=f"{tag}_u3")
    engA.tensor_mul(u3, u2, u)
    tick()
    w1 = pool.tile([128, W], F16, tag=f"{tag}_w1", name=f"{tag}_w1")
    engA.tensor_scalar(out=w1, in0=u, scalar1=-1.0, scalar2=1.0,
                       op0=ALU.mult, op1=ALU.add)
    w2 = pool.tile([128, W], F16, tag=f"{tag}_w2", name=f"{tag}_w2")
    engA.tensor_mul(w2, w1, w1)
    D3 = pool.tile([128, W], F16, tag=f"{tag}_D3", name=f"{tag}_D3")
    engA.tensor_mul(D3, w2, w1)
    D0 = u3
    u26 = pool.tile([128, W], F16, tag=f"{tag}_u26", name=f"{tag}_u26")
    engA.tensor_scalar_mul(u26, u2, 6.0)
    D2 = pool.tile([128, W], F16, tag=f"{tag}_D2", name=f"{tag}_D2")
    nc.vector.scalar_tensor_tensor(out=D2, in0=u3, scalar=3.0, in1=u26,
                                   op0=ALU.mult, op1=ALU.subtract)
    engA.tensor_scalar_add(D2, D2, 4.0)
    tick()
    D1 = pool.tile([128, W], F16, tag=f"{tag}_D1", name=f"{tag}_D1")
    engA.tensor_add(D1, D0, D2)
    engA.tensor_add(D1, D1, D3)
    engA.tensor_scalar(out=D1, in0=D1, scalar1=-1.0, scalar2=6.0,
                       op0=ALU.mult, op1=ALU.add)
    Ds = [D0, D1, D2, D3]
    ms = []
    for j in range(4):
        m = pool.tile([128, 6, W], F16, tag=f"{tag}_m{j}", name=f"{tag}_m{j}")
        nc.vector.tensor_mul(m, q[:, j:j + 6],
                             Ds[j][:, None, :].broadcast_to([128, 6, W]))
        ms.append(m)
    tick()
    a01 = pool.tile([128, 6, W], F16, tag=f"{tag}_a01", name=f"{tag}_a01")
    engA.tensor_add(a01, ms[0], ms[1])
    a23 = pool.tile([128, 6, W], F16, tag=f"{tag}_a23", name=f"{tag}_a23")
    engB.tensor_add(a23, ms[2], ms[3])
    bs = pool.tile([128, 6, W], BF16, tag=f"{tag}_bs", name=f"{tag}_bs")
    engA.tensor_add(bs, a01, a23)
    return bs


def _build_program():
    nc = bacc.Bacc("TRN2", target_bir_lowering=False, debug=False,
                   num_devices=NCORES)
    xpad_d = nc.dram_tensor("xpad", [C, BC, LP], BF16,
                            kind="ExternalInput").ap()
    wconv_d = nc.dram_tensor("wconv", [WCONV_ROWS, 128], BF16,
                             kind="ExternalInput").ap()
    kconst_d = nc.dram_tensor("kconst", [128, 5, 96], F32,
                              kind="ExternalInput").ap()
    kvec_d = nc.dram_tensor("kvec", [128, 10], F16, kind="ExternalInput").ap()
    w1s_d = nc.dram_tensor("w1s", [128, NW1, D1OUT], BF16,
                           kind="ExternalInput").ap()
    w2s_d = nc.dram_tensor("w2s", [128, NW2, W2PAD], BF16,
                           kind="ExternalInput").ap()
    id32_d = nc.dram_tensor("id32", [32, 32], F32, kind="ExternalInput").ap()
    out_d = nc.dram_tensor("out", [BC, D2OUT], F32, kind="ExternalOutput").ap()

    with tile.TileContext(nc) as tc:
        with (
            tc.tile_pool(name="const", bufs=1) as const,
            tc.tile_pool(name="patches", bufs=2) as patches,
            tc.tile_pool(name="work", bufs=1) as work,
            tc.tile_pool(name="psconv", bufs=1, space="PSUM") as psconv,
            tc.tile_pool(name="psc1", bufs=3, space="PSUM") as psc1,
            tc.tile_pool(name="pstail", bufs=1, space="PSUM") as pstail,
        ):
            # ---- constants / warmup ----
            wu_l = const.tile([128, 128], BF16, tag="wu_l", name="wu_l")
            nc.vector.memset(wu_l, 0.125)
            wu_r = const.tile([128, 512], BF16, tag="wu_r", name="wu_r")
            nc.vector.memset(wu_r, 0.125)
            ones = const.tile([128, 128], F32, tag="ones", name="ones")
            nc.vector.memset(ones, 1.0)
            # HAM warm-up: keep PE busy while the first patch DMAs land.
            wups = pstail.tile([128, 512], F32, tag="big", name="wups")
            for i in range(10):
                nc.tensor.matmul(out=wups, lhsT=wu_l, rhs=wu_r,
                                 start=True, stop=True)
            # pre-load the ACT table set for Sigmoid (the only ACT function
            # used; Copy needs no table)
            wtab = const.tile([128, 1], F32, tag="wtab", name="wtab")
            nc.scalar.activation(out=wtab, in_=ones[:, 0:1], func=ACTF.Sigmoid)

            wc_tiles = []
            for gi, (r0, nr, _off, _cj, _f, _l) in enumerate(GROUPS):
                wt = const.tile([128, 128], BF16, tag=f"wc{gi}", name=f"wc{gi}")
                nc.sync.dma_start(out=wt[0:nr, :], in_=wconv_d[r0:r0 + nr, :])
                wc_tiles.append(wt)
            kc = const.tile([128, 5, 96], F32, tag="kc", name="kc")
            nc.sync.dma_start(out=kc, in_=kconst_d)
            kvec = work.tile([128, 10], F16, tag="kvec", name="kvec_l")
            nc.sync.dma_start(out=kvec, in_=kvec_d)
            id32 = const.tile([32, 32], F32, tag="id32", name="id32")
            nc.sync.dma_start(out=id32, in_=id32_d)
            w1s = const.tile([128, NW1, D1OUT], BF16, tag="w1s", name="w1s")
            nc.sync.dma_start(out=w1s, in_=w1s_d)
            w2s = const.tile([128, NW2, W2PAD], BF16, tag="w2s", name="w2s")
            nc.sync.dma_start(out=w2s, in_=w2s_d)

            # mraw rows: conv1-half-a, conv1-half-b, conv2, conv3
            mraw = work.tile([128, 4, BC], F32, tag="mraw", name="mraw")
            kc3 = kc.rearrange("p i (j b) -> p i j b", j=3)
            # conv2+conv3 maxpool: ACT casts PSUM->SBUF fp16 into a padded
            # [128, 2, 1024] tile; DVE folds both at 2x fp16 rate into a
            # per-chunk buffer; one batched fold+reduce per chunk.
            # conv1 is reduced straight from PSUM on DVE in two phases
            # (two single-bank psum generations -> no PE<->DVE convoy).
            cpBs = []
            for par in range(2):
                cpB = work.tile([128, 2, 1024], F16, tag=f"cpB{par}",
                                name=f"cpB{par}")
                nc.vector.memset(cpB, NEG_INF)
                cpBs.append(cpB)
            ckbufs = []
            for par in range(2):
                ckb = work.tile([128, 3, CH, 256], F16, tag=f"ckb{par}",
                                name=f"ckb{par}")
                ckbufs.append(ckb)

            def emit_tail_stages(b0, hx):
                """Return [s0..s3] closures for the tail over samples
                [b0, b0+NH). hx selects engine balance (0: conv still
                running -> gpsimd-heavy; 1: end of kernel -> mixed)."""
                nb = NH
                W1W = 3 * nb
                W2W = 4 * nb
                sfx = f"h{hx}"
                # tail1 overlaps the conv stream: put the affine chain on
                # GPSIMD and split spline work DVE/GPSIMD. tail2 runs at
                # the end when DVE is free: everything on DVE for minimum
                # chain latency.
                ge = nc.gpsimd if hx == 0 else nc.vector
                engB = nc.gpsimd if hx == 0 else nc.vector
                engA = nc.vector
                st_ = {}

                def tick():
                    # dependency-bound LDWEIGHTS: executes as the spline
                    # chain progresses, keeping HAM warm through the tail-2
                    # end chain without real matmul work
                    if hx == 1:
                        nc.tensor.ldweights(weights=wu_l)

                def keepwarm(n):
                    # dummy matmuls keep HAM at 8/8 while the PE waits for
                    # the tail's DVE/GPSIMD chains (tail 2 only; during
                    # tail 1 the conv stream keeps the PE busy)
                    if hx == 0:
                        return
                    wk = pstail.tile([128, 64], F32, tag="big",
                                     name=f"wk{sfx}")
                    for _ in range(n):
                        nc.tensor.matmul(out=wk, lhsT=wu_l,
                                         rhs=wu_r[:, 0:64],
                                         start=True, stop=True)

                def s_aff():
                    # merge conv1's two partial maxima into row 1 in place
                    nc.vector.tensor_max(mraw[:, 1, b0:b0 + nb],
                                         mraw[:, 0, b0:b0 + nb],
                                         mraw[:, 1, b0:b0 + nb])
                    mrh = mraw[:, 1:4, b0:b0 + nb]
                    kch = kc3[:, :, :, b0:b0 + nb]
                    h96 = work.tile([128, 3, nb], F32, tag=f"h96{sfx}",
                                    name=f"h96{sfx}")
                    ge.tensor_add(h96, mrh, kch[:, 0])
                    ge.tensor_scalar_max(h96, h96, 0.0)
                    ge.tensor_mul(h96, h96, kch[:, 1])
                    ge.tensor_add(h96, h96, kch[:, 2])
                    sq96 = work.tile([128, 3, nb], F32, tag=f"sq96{sfx}",
                                     name=f"sq96{sfx}")
                    ge.tensor_mul(sq96, h96, h96)
                    st_["h96"] = h96
                    st_["sq96"] = sq96
                    st_["kch"] = kch
                    keepwarm(4)

                def s_ln():
                    h96, sq96, kch = st_["h96"], st_["sq96"], st_["kch"]
                    psLN = pstail.tile([1, 4 * W1W], F32, tag="big",
                                       name=f"psLN{sfx}")
                    nc.tensor.matmul(out=psLN[0:1, 0:W1W], lhsT=ones[:, 0:1],
                                     rhs=h96, start=True, stop=True)
                    nc.tensor.matmul(out=psLN[0:1, W1W:2 * W1W],
                                     lhsT=ones[:, 0:1], rhs=sq96,
                                     start=True, stop=True)
                    sums = work.tile([1, 2, nb], F32, tag=f"sums{sfx}",
                                     name=f"sums{sfx}")
                    psLNv = psLN[0:1, 0:2 * W1W].rearrange(
                        "p (x j b) -> p x b j", x=2, j=3)
                    nc.vector.reduce_sum(out=sums[0:1, 0], in_=psLNv[0:1, 0],
                                         axis=AX.X)
                    nc.vector.reduce_sum(out=sums[0:1, 1], in_=psLNv[0:1, 1],
                                         axis=AX.X)
                    muinv = work.tile([1, 2, nb], F32, tag=f"muinv{sfx}",
                                      name=f"muinv{sfx}")
                    nc.vector.tensor_scalar_mul(muinv[0:1, 0], sums[0:1, 0],
                                                1.0 / 384)
                    msq = work.tile([1, nb], F32, tag=f"msq{sfx}",
                                    name=f"msq{sfx}")
                    nc.vector.tensor_mul(msq, muinv[0:1, 0], muinv[0:1, 0])
                    var = work.tile([1, nb], F32, tag=f"var{sfx}",
                                    name=f"var{sfx}")
                    nc.vector.scalar_tensor_tensor(out=var, in0=sums[0:1, 1],
                                                   scalar=1.0 / 384, in1=msq,
                                                   op0=ALU.mult,
                                                   op1=ALU.subtract)
                    nc.vector.tensor_scalar_add(var, var, 1e-5)
                    # rsqrt(var) via bit-trick + 2 Newton steps, all on DVE
                    # (ACT Sqrt would force a table-set reload between the
                    # tail's Sigmoid uses: ~2.7us each on the critical path)
                    yi = work.tile([1, nb], mybir.dt.int32, tag=f"yi{sfx}",
                                   name=f"yi{sfx}")
                    nc.vector.tensor_scalar(out=yi, in0=var.bitcast(
                        mybir.dt.int32), scalar1=1, scalar2=None,
                        op0=ALU.arith_shift_right)
                    nc.vector.tensor_scalar(out=yi, in0=yi, scalar1=-1,
                                            scalar2=0x5f3759df, op0=ALU.mult,
                                            op1=ALU.add)
                    y = work.tile([1, nb], F32, tag=f"y{sfx}",
                                  name=f"y{sfx}")
                    nc.vector.tensor_copy(y, yi.bitcast(F32))
                    vh = work.tile([1, nb], F32, tag=f"vh{sfx}",
                                   name=f"vh{sfx}")
                    nc.vector.tensor_scalar_mul(vh, var, 0.5)
                    yt = work.tile([1, nb], F32, tag=f"yt{sfx}",
                                   name=f"yt{sfx}")
                    for _ in range(2):
                        nc.vector.tensor_mul(yt, y, y)
                        nc.vector.tensor_mul(yt, yt, vh)
                        nc.vector.tensor_scalar(out=yt, in0=yt, scalar1=-1.0,
                                                scalar2=1.5, op0=ALU.mult,
                                                op1=ALU.add)
                        nc.vector.tensor_mul(y, y, yt)
                    nc.vector.tensor_copy(muinv[0:1, 1], y)
                    psB = pstail.tile([128, 2, nb], F32, tag="big",
                                      name=f"psB{sfx}")
                    nc.tensor.matmul(out=psB, lhsT=ones[0:1, :],
                                     rhs=muinv[0:1], start=True, stop=True)
                    muinvB = work.tile([128, 2, nb], F32, tag=f"muinvB{sfx}",
                                       name=f"muinvB{sfx}")
                    nc.scalar.copy(out=muinvB, in_=psB)
                    hn = work.tile([128, 3, nb], F32, tag=f"hn{sfx}",
                                   name=f"hn{sfx}")
                    nc.vector.tensor_sub(hn, h96,
                                         muinvB[:, 0, None, :].broadcast_to(
                                             [128, 3, nb]))
                    nc.vector.tensor_mul(hn, hn,
                                         muinvB[:, 1, None, :].broadcast_to(
                                             [128, 3, nb]))
                    ge.tensor_mul(hn, hn, kch[:, 3])
                    ge.tensor_add(hn, hn, kch[:, 4])
                    st_["hn"] = hn
                    keepwarm(6)

                def s_silu1():
                    hn = st_["hn"]
                    hn2d = hn.rearrange("p j b -> p (j b)")
                    sig = work.tile([128, W1W], F32, tag=f"sig{sfx}",
                                    name=f"sig{sfx}")
                    nc.scalar.activation(out=sig, in_=hn2d, func=ACTF.Sigmoid)
                    sil = work.tile([128, W1W], BF16, tag=f"sil{sfx}",
                                    name=f"sil{sfx}")
                    engA.tensor_mul(sil, sig, hn2d)
                    bs1 = _emit_splines(nc, work, hn2d, W1W, f"sp1{sfx}",
                                        engA, engB, kvec, tick)
                    st_["sil"] = sil
                    st_["bs1"] = bs1
                    keepwarm(14)

                def s_kan1():
                    sil, bs1 = st_["sil"], st_["bs1"]
                    psK1 = pstail.tile([nb, D1OUT], F32, tag="big",
                                       name=f"psK1{sfx}")
                    mi = 0
                    for j in range(3):
                        nc.tensor.matmul(out=psK1,
                                         lhsT=sil[:, j * nb:(j + 1) * nb],
                                         rhs=w1s[:, j], start=(mi == 0),
                                         stop=(mi == NW1 - 1))
                        mi += 1
                    for j in range(3):
                        for g in range(6):
                            nc.tensor.matmul(
                                out=psK1,
                                lhsT=bs1[:, g, j * nb:(j + 1) * nb],
                                rhs=w1s[:, 3 + j * 6 + g],
                                start=(mi == 0), stop=(mi == NW1 - 1))
                            mi += 1
                    h2s = work.tile([nb, D1OUT], F32, tag=f"h2s{sfx}",
                                    name=f"h2s{sfx}")
                    nc.scalar.copy(out=h2s, in_=psK1)
                    psT = pstail.tile([128, 4 * nb], F32, tag="big",
                                      name=f"psT{sfx}")
                    for j in range(4):
                        nc.tensor.transpose(out=psT[:, j * nb:(j + 1) * nb],
                                            in_=h2s[:, j * 128:(j + 1) * 128],
                                            identity=id32[0:nb, 0:nb])
                    h2T = work.tile([128, 4 * nb], F32, tag=f"h2T{sfx}",
                                    name=f"h2T{sfx}")
                    nc.scalar.copy(out=h2T, in_=psT)
                    st_["h2T"] = h2T
                    keepwarm(4)

                def s_silu2():
                    h2T = st_["h2T"]
                    sig2 = work.tile([128, W2W], F32, tag=f"sig2{sfx}",
                                     name=f"sig2{sfx}")
                    nc.scalar.activation(out=sig2, in_=h2T, func=ACTF.Sigmoid)
                    sil2 = work.tile([128, W2W], BF16, tag=f"sil2{sfx}",
                                     name=f"sil2{sfx}")
                    engA.tensor_mul(sil2, sig2, h2T)
                    bs2 = _emit_splines(nc, work, h2T, W2W, f"sp2{sfx}",
                                        engA, engB, kvec, tick)
                    st_["sil2"] = sil2
                    st_["bs2"] = bs2
                    keepwarm(14)

                def s_kan2():
                    sil2, bs2 = st_["sil2"], st_["bs2"]
                    psK2 = pstail.tile([nb, W2PAD], F32, tag="big",
                                       name=f"psK2{sfx}")
                    mi = 0
                    for j in range(4):
                        nc.tensor.matmul(out=psK2,
                                         lhsT=sil2[:, j * nb:(j + 1) * nb],
                                         rhs=w2s[:, j], start=(mi == 0),
                                         stop=(mi == NW2 - 1))
                        mi += 1
                    for j in range(4):
                        for g in range(6):
                            nc.tensor.matmul(
                                out=psK2,
                                lhsT=bs2[:, g, j * nb:(j + 1) * nb],
                                rhs=w2s[:, 4 + j * 6 + g],
                                start=(mi == 0), stop=(mi == NW2 - 1))
                            mi += 1
                    outS = work.tile([nb, D2OUT], F32, tag=f"outS{sfx}",
                                     name=f"outS{sfx}")
                    nc.scalar.copy(out=outS, in_=psK2[:, 0:D2OUT])
                    nc.sync.dma_start(out=out_d[b0:b0 + nb], in_=outS)

                return [s_aff, s_ln, s_silu1, s_kan1, s_silu2, s_kan2]

            tail1 = emit_tail_stages(0, 0)
            tail2 = emit_tail_stages(NH, 1)
            # (chunk, sample) emission points for tail1's stages, spaced so
            # each stage's inputs are ready when the fenced PE stream
            # reaches its matmuls
            t1_points = {(5, 0): tail1[0], (5, 2): tail1[1],
                         (6, 0): tail1[2], (6, 3): tail1[3],
                         (7, 1): tail1[4], (7, 3): tail1[5]}

            # ---- conv phase ----
            # deferred per-chunk fold/reduce closures, drained one per
            # sample so the DVE never sees a chunk-boundary burst
            pending_ck = []

            def emit_ck_stage(c):
                ckb = ckbufs[c % 2]
                fA = work.tile([128, 3, CH, 128], F16, tag="fA", name="fA")
                fB = work.tile([128, 3, CH, 64], F16, tag="fB", name="fB")
                fC = work.tile([128, 3, CH, 32], F16, tag="fC", name="fC")
                pending_ck.append(lambda: nc.vector.tensor_max(
                    fA, ckb[:, :, :, 0:128], ckb[:, :, :, 128:256]))
                pending_ck.append(lambda: nc.vector.tensor_max(
                    fB, fA[:, :, :, 0:64], fA[:, :, :, 64:128]))
                pending_ck.append(lambda: nc.vector.tensor_max(
                    fC, fB[:, :, :, 0:32], fB[:, :, :, 32:64]))
                pending_ck.append(lambda: nc.vector.reduce_max(
                    out=mraw[:, 1:4, c * CH:(c + 1) * CH].unsqueeze(-1),
                    in_=fC, axis=AX.X))

            for c in range(NCHUNK):
                ptile = patches.tile([128, CH, LP], BF16, name="ptile")
                for s in range(6):
                    nc.gpsimd.dma_start(
                        out=ptile[s * C:(s + 1) * C, :, 0:LP - s],
                        in_=xpad_d[:, c * CH:(c + 1) * CH, s:LP])
                ckb = ckbufs[c % 2]
                for bi in range(CH):
                    stage = t1_points.get((c, bi))
                    if stage is not None:
                        stage()
                    b = c * CH + bi
                    pc = [None,
                          psconv.tile([128, 1024], F32, tag="pc1",
                                      name="pc1"),
                          psconv.tile([128, 1024], F32, tag="pc2",
                                      name="pc2")]
                    # conv1 phases first (their DVE reduces are the only
                    # PSUM->DVE drains; issuing them early gives the
                    # conv1 banks ~3 sample periods of slack), conv3
                    # before conv2 (its ACT copy starts first)
                    pc0a = psc1.tile([128, 512], F32, name="pc0")
                    nc.tensor.matmul(out=pc0a, lhsT=wc_tiles[0][0:84, :],
                                     rhs=ptile[0:84, bi, 0:512],
                                     start=True, stop=True)
                    pc0b = psc1.tile([128, 512], F32, name="pc0")
                    nc.tensor.matmul(out=pc0b[:, 0:485],
                                     lhsT=wc_tiles[0][0:84, :],
                                     rhs=ptile[0:84, bi, 512:CONV_L[0]],
                                     start=True, stop=True)
                    for gi in (3, 4, 5, 1, 2):
                        r0, nr, off, cj, first, last = GROUPS[gi]
                        for (n0, n1) in ((0, 512), (512, CONV_L[cj])):
                            nc.tensor.matmul(
                                out=pc[cj][:, n0:n1],
                                lhsT=wc_tiles[gi][0:nr, :],
                                rhs=ptile[0:nr, bi, off + n0:off + n1],
                                start=first, stop=last,
                            )
                    nc.vector.reduce_max(out=mraw[:, 0, b:b + 1],
                                         in_=pc0b[:, 0:485], axis=AX.X)
                    if c == 0:
                        nc.vector.reduce_max(out=mraw[:, 1, b:b + 1],
                                             in_=pc0a, axis=AX.X)
                    else:
                        pa = work.tile([128, 512], F16, tag=f"pa{b % 2}",
                                       name=f"pa{b % 2}")
                        nc.scalar.copy(out=pa, in_=pc0a)
                        nc.vector.tensor_max(ckb[:, 0, bi, :], pa[:, 0:256],
                                             pa[:, 256:512])
                    if c == 0:
                        # startup: ACT is loading tables; drain conv2/3
                        # straight on the (idle) DVE instead
                        nc.vector.reduce_max(out=mraw[:, 2, b:b + 1],
                                             in_=pc[1][:, 0:CONV_L[1]],
                                             axis=AX.X)
                        nc.vector.reduce_max(out=mraw[:, 3, b:b + 1],
                                             in_=pc[2][:, 0:CONV_L[2]],
                                             axis=AX.X)
                    else:
                        cpB = cpBs[b % 2]
                        nc.scalar.copy(out=cpB[:, 1, 0:CONV_L[2]],
                                       in_=pc[2][:, 0:CONV_L[2]])
                        nc.scalar.copy(out=cpB[:, 0, 0:CONV_L[1]],
                                       in_=pc[1][:, 0:CONV_L[1]])
                        fB1 = work.tile([128, 2, 512], F16, tag="fB1",
                                        name="fB1")
                        nc.vector.tensor_max(fB1, cpB[:, :, 0:512],
                                             cpB[:, :, 512:1024])
                        nc.vector.tensor_max(ckb[:, 1:3, bi, :],
                                             fB1[:, :, 0:256],
                                             fB1[:, :, 256:512])
                    if pending_ck:
                        pending_ck.pop(0)()
                    # scheduler-only fence: keep the per-sample interleaving
                    # (without it the scheduler clusters same-kind matmuls
                    # at chunk boundaries and ping-pongs PE<->DVE)
                    tc.no_sync_barrier()
                if c > 0:
                    emit_ck_stage(c)
            while pending_ck:
                pending_ck.pop(0)()
            for stage in tail2:
                tc.no_sync_barrier()
                stage()
    nc.compile()
    return nc


def _host_prep(inputs):
    f = np.float32
    bf = ml_dtypes.bfloat16
    x = np.asarray(inputs["x"], f)
    xT = np.ascontiguousarray(x.transpose(0, 2, 1))  # [B, 21, 1000]
    xTpad = np.zeros((B, C, LP), f)
    xTpad[:, :, :L] = xT
    xpads = []
    for i in range(NCORES):
        sh = xTpad[i * BC:(i + 1) * BC]  # [BC, 21, LP]
        xpads.append(np.ascontiguousarray(
            sh.transpose(1, 0, 2)).astype(bf))  # [21, BC, LP]

    def chunks(w, taps):
        return [np.ascontiguousarray(
            np.asarray(w, f)[:, :, t0:t1].transpose(2, 1, 0).reshape((t1 - t0) * C, 128))
            for t0, t1 in taps]

    wconv = np.concatenate(
        chunks(inputs["conv1_w"], [(0, 4)])
        + chunks(inputs["conv2_w"], [(0, 6), (6, 8)])
        + chunks(inputs["conv3_w"], [(0, 6), (6, 12), (12, 16)]), 0)

    def fold(p):
        g, bb, m, v = (np.asarray(inputs[p + s], f) for s in ("_g", "_b", "_m", "_v"))
        s = g / np.sqrt(v + 1e-5)
        return s, bb - m * s

    s1, t1 = fold("bn1")
    s2, t2 = fold("bn2")
    s3, t3 = fold("bn3")
    s4, t4 = fold("bn4")
    Sall = np.concatenate([s1, s2, s3]) * s4
    Tall = np.concatenate([t1, t2, t3]) * s4 + t4
    cb = np.concatenate([np.asarray(inputs["conv1_b"], f),
                         np.asarray(inputs["conv2_b"], f),
                         np.asarray(inputs["conv3_b"], f)])

    def expand(v):
        return np.repeat(np.asarray(v, f).reshape(3, 128).T[:, :, None], BC, 2)

    kconst = np.stack([expand(cb), expand(Sall), expand(Tall),
                       expand(np.asarray(inputs["ln_g"], f)),
                       expand(np.asarray(inputs["ln_b"], f))], 1)
    kconst = np.ascontiguousarray(kconst.reshape(128, 5, 96))

    kvec = np.broadcast_to(np.arange(10, dtype=f)[None, :], (128, 10))

    bw1 = np.asarray(inputs["base_w1"], f)
    sw1 = np.asarray(inputs["spline_w1"], f) / 6.0
    w1s = np.empty((128, NW1, D1OUT), f)
    for j in range(3):
        w1s[:, j, :] = bw1[:, j * 128:(j + 1) * 128].T
        for g in range(6):
            w1s[:, 3 + j * 6 + g, :] = sw1[:, j * 128:(j + 1) * 128, g].T
    bw2 = np.asarray(inputs["base_w2"], f)
    sw2 = np.asarray(inputs["spline_w2"], f) / 6.0
    w2s = np.zeros((128, NW2, W2PAD), f)
    for j in range(4):
        w2s[:, j, :D2OUT] = bw2[:, j * 128:(j + 1) * 128].T
        for g in range(6):
            w2s[:, 4 + j * 6 + g, :D2OUT] = sw2[:, j * 128:(j + 1) * 128, g].T

    shared = {
        "wconv": np.ascontiguousarray(wconv).astype(bf),
        "kconst": kconst,
        "kvec": np.ascontiguousarray(kvec).astype(np.float16),
        "w1s": np.ascontiguousarray(w1s).astype(bf),
        "w2s": np.ascontiguousarray(w2s).astype(bf),
        "id32": np.eye(32, dtype=f),
    }
    return shared, xpads


_NC_CACHE = None


def _get_nc():
    global _NC_CACHE
    if _NC_CACHE is None:
        _NC_CACHE = _build_program()
    return _NC_CACHE


def make_in_maps(inputs):
    shared, xpads = _host_prep(inputs)
    return [{**shared, "xpad": xpads[i]} for i in range(NCORES)]


def kernel(**inputs):
    nc = _get_nc()
    in_maps = make_in_maps(inputs)
    res = run_bass_kernel_spmd(nc, in_maps, list(range(NCORES)))
    return np.concatenate([res.results[i]["out"] for i in range(NCORES)], 0)
